# revision 1
# baseline (speedup 1.0000x reference)
"""V3: pencil-decomposition 2D DCT-II with on-device basis generation.

Rationale: the grader's wall-clock is dominated by host<->device traffic
(~20GB/s through the axon tunnel).  The V2 kernel shipped 864MB (full folded
x + stage-2 basis replicated to all 8 cores).  V3 ships only each core's
512-row shard of x in bf16 (4MB/core) plus ~200KB of generator vectors, and
returns bf16 output shards (4MB/core): ~66MB total.

Per-core dataflow (identical SPMD program, core identity implicit in data):
  stage 1: Y[k, col] = sum_a W1[a, k] * xf[a + 2048*(k%2), col]
           for all k in [0,4096), col = this core's 512 (paired) x-rows.
           W1[a, k] = A1[k, a] is generated on device:
             v = j_lo[a]*k + j_hi[a]*((64k)%N) + (5120 - f_k)   (3 one-row
                 PE matmuls accumulated in PSUM, exact in f32)
             W = sin((2pi/N)*(v mod N) - pi) = -(A1[k,a]/R_k)   (DVE mod,
                 ACT Sin); the -R_k scale is applied at the PSUM-copy.
  A2A:     DRAM AllToAll redistributes Y so each core holds its contiguous
           512-wide k-slice for all 4096 columns.
  stage 2: fold column pairs (r, 4095-r) -> sum/diff, PE-transpose to put
           the fold index on partitions, then same generated-basis matmul
           with A0, writing bf16 out[l, k-slice].
"""
import numpy as np
import ml_dtypes

BF16 = ml_dtypes.bfloat16
N = 4096
H = N // 2
P = 128
KC = 512
NCORES = 8

_NC_CACHE = {}


def _perm_inv(a):
    a = np.asarray(a)
    return np.where(a % 2 == 0, a // 2, N - 1 - (a - 1) // 2)


def _gen_side(expk):
    """Tiny per-stage generator vectors (f32) from expk [N, 2]."""
    c = expk[:, 0].astype(np.float64)
    s = expk[:, 1].astype(np.float64)
    R = np.hypot(c, s)
    f = np.arctan2(s, c) * N / (2 * np.pi)
    fv = 5120.0 - f
    k = np.arange(N, dtype=np.float64)
    k64m = (64.0 * k) % N
    g = np.stack([k[0::2], k[1::2], k64m[0::2], k64m[1::2],
                  fv[0::2], fv[1::2]]).astype(np.float32)   # [6, 2048]
    # -R arranged [128, 32] so column (par*16 + grp*4 + t) row p holds
    # -R[2*(512*grp + 128*t + p) + par]
    r = np.empty((P, 32), np.float32)
    for par in range(2):
        for grp in range(4):
            for t in range(4):
                kk = 2 * (512 * grp + P * t + np.arange(P)) + par
                r[:, par * 16 + grp * 4 + t] = -R[kk]
    return g, r


def _prep(x, expk0, expk1):
    x = np.asarray(x, np.float32)
    g1, r1 = _gen_side(np.asarray(expk1, np.float32))
    g0, r0 = _gen_side(np.asarray(expk0, np.float32))
    j = _perm_inv(np.arange(H))
    jj = np.stack([j % 64, j // 64]).astype(np.float32)     # [2, 2048]
    eye = np.eye(P, dtype=BF16)
    one = np.ones((1, P), np.float32)

    xrev = x[:, ::-1][:, :H]
    xs = x[:, :H] + xrev
    xd = x[:, :H] - xrev
    common = {"g1": g1, "r1": r1, "g0": g0, "r0": r0, "jj": jj,
              "eye": eye, "one": one}
    in_maps = []
    for c in range(NCORES):
        t = np.arange(256 * c, 256 * (c + 1))
        rows = np.empty(512, np.int64)
        rows[0::2] = t
        rows[1::2] = N - 1 - t
        xf = np.concatenate([xs[rows].T, xd[rows].T], axis=0)  # [4096, 512]
        in_maps.append({"xf": np.ascontiguousarray(xf.astype(BF16)), **common})
    return in_maps


def _build_nc(reps=1):
    import concourse.bacc as bacc
    import concourse.mybir as mybir
    import concourse.tile as tile

    FP32 = mybir.dt.float32
    BF = mybir.dt.bfloat16
    SIN = mybir.ActivationFunctionType.Sin
    I32 = mybir.dt.int32
    AND = mybir.AluOpType.bitwise_and
    MULT = mybir.AluOpType.mult
    ADD = mybir.AluOpType.add
    SUB = mybir.AluOpType.subtract
    SC = float(2.0 * np.pi / N)
    BIAS = float(-np.pi)

    nc = bacc.Bacc("TRN2", target_bir_lowering=False, debug=False,
                   num_devices=NCORES)

    xf_d = nc.dram_tensor("xf", [N, KC], BF, kind="ExternalInput")
    g1_d = nc.dram_tensor("g1", [6, H], FP32, kind="ExternalInput")
    r1_d = nc.dram_tensor("r1", [P, 32], FP32, kind="ExternalInput")
    g0_d = nc.dram_tensor("g0", [6, H], FP32, kind="ExternalInput")
    r0_d = nc.dram_tensor("r0", [P, 32], FP32, kind="ExternalInput")
    jj_d = nc.dram_tensor("jj", [2, H], FP32, kind="ExternalInput")
    eye_d = nc.dram_tensor("eye", [P, P], BF, kind="ExternalInput")
    one_d = nc.dram_tensor("one", [1, P], FP32, kind="ExternalInput")
    out_d = nc.dram_tensor("out", [H, 2, KC], BF, kind="ExternalOutput")

    with tile.TileContext(nc) as tc:
      for _rep in range(reps):
        with tc.tile_pool(name="const", bufs=1) as cpool:
            r1 = cpool.tile([P, 32], FP32)
            nc.sync.dma_start(r1[:], r1_d[:])
            r0 = cpool.tile([P, 32], FP32)
            nc.sync.dma_start(r0[:], r0_d[:])
            jj = [[cpool.tile([1, P], FP32, name=f'jj_{i}_{ac}')
                   for ac in range(16)] for i in range(2)]
            for i in range(2):
                for ac in range(16):
                    nc.sync.dma_start(
                        jj[i][ac][:], jj_d[i:i + 1, ac * P:(ac + 1) * P])
            eye = cpool.tile([P, P], BF)
            nc.sync.dma_start(eye[:], eye_d[:])
            one = cpool.tile([1, P], FP32)
            nc.sync.dma_start(one[:], one_d[:])
            bias_t = cpool.tile([P, 1], FP32)
            nc.vector.memset(bias_t[:], BIAS)

            with tc.tile_pool(name="dram", bufs=2, space="DRAM") as dram:
                # [q, par, c] with row k = 2q+par: same memory as [N, KC]
                bi = dram.tile([H, 2, KC], BF)
                # bo layout after A2A: [src s, kt, p, u, two] where the
                # received flat row 512*s + 128*kt + p holds Y[k=my slice
                # local 128*kt+p] from src s, columns c = 2*u + two
                bo = dram.tile([NCORES, 4, P, 256, 2], BF)

                # ----- stage 1 -----
                with (
                    tc.tile_pool(name="xfp", bufs=1) as xfpool,
                    tc.tile_pool(name="gsl", bufs=2) as gpool,
                    tc.tile_pool(name="wtmp", bufs=3) as wtpool,
                    tc.tile_pool(name="wpool", bufs=3) as wpool,
                    tc.tile_pool(name="ypool", bufs=4) as ypool,
                    tc.tile_pool(name="psy", bufs=1, space="PSUM") as psy,
                    tc.tile_pool(name="psw", bufs=2, space="PSUM") as psw,
                ):
                    xf = xfpool.tile([P, 32, KC], BF)
                    nc.sync.dma_start(
                        xf[:], xf_d[:].rearrange("(ch p) c -> p ch c", p=P))
                    for par in range(2):
                        for g in range(4):
                            ksl = slice(g * KC, (g + 1) * KC)
                            gk = gpool.tile([1, KC], FP32, name='gk')
                            gm = gpool.tile([1, KC], FP32, name='gm')
                            gf = gpool.tile([1, KC], FP32, name='gf')
                            nc.sync.dma_start(gk[:], g1_d[par:par + 1, ksl])
                            nc.sync.dma_start(
                                gm[:], g1_d[2 + par:3 + par, ksl])
                            nc.sync.dma_start(
                                gf[:], g1_d[4 + par:5 + par, ksl])
                            py = [psy.tile([P, KC], FP32, name=f'py{kt}')
                                  for kt in range(4)]
                            for ac in range(16):
                                pw = psw.tile([P, KC], FP32)
                                nc.tensor.matmul(
                                    pw[:], jj[0][ac][:], gk[:],
                                    start=True, stop=False)
                                nc.tensor.matmul(
                                    pw[:], jj[1][ac][:], gm[:],
                                    start=False, stop=False)
                                nc.tensor.matmul(
                                    pw[:], one[:], gf[:],
                                    start=False, stop=True)
                                wi = wtpool.tile([P, KC], I32, name='wi')
                                nc.vector.tensor_copy(wi[:], pw[:])
                                wi2 = wtpool.tile([P, KC], I32, name='wi2')
                                nc.vector.tensor_scalar(
                                    wi2[:], wi[:], N - 1, None, AND)
                                wb = wpool.tile([P, KC], BF)
                                nc.scalar.activation(
                                    wb[:], wi2[:], SIN, bias=bias_t[:],
                                    scale=SC)
                                mv = xf[:, par * 16 + ac, :]
                                for kt in range(4):
                                    nc.tensor.matmul(
                                        py[kt][:], wb[:, kt * P:(kt + 1) * P],
                                        mv, start=(ac == 0), stop=(ac == 15))
                            for kt in range(4):
                                yb = ypool.tile([P, KC], BF)
                                idx = par * 16 + g * 4 + kt
                                nc.vector.tensor_scalar(
                                    yb[:], py[kt][:], r1[:, idx:idx + 1],
                                    None, MULT)
                                q0 = KC * g + P * kt
                                nc.sync.dma_start(
                                    bi[q0:q0 + P, par, :], yb[:])

                # ----- all-to-all -----
                nc.gpsimd.collective_compute(
                    "AllToAll", mybir.AluOpType.bypass,
                    replica_groups=[list(range(NCORES))],
                    ins=[bi.opt()], outs=[bo.opt()])

                # ----- stage 2: gather + fold + transpose -----
                with tc.tile_pool(name="yst", bufs=1) as ystp:
                    ysT = ystp.tile([P, 16, KC], BF)
                    ydT = ystp.tile([P, 16, KC], BF)
                    with (
                        tc.tile_pool(name="gt", bufs=1) as gtp,
                        tc.tile_pool(name="ft", bufs=2) as ftp,
                        tc.tile_pool(name="pst", bufs=2, space="PSUM") as pst,
                    ):
                        yt = gtp.tile([P, 4, NCORES, 256, 2], BF)
                        for kt in range(4):
                            nc.sync.dma_start(
                                yt[:, kt, :, :, :],
                                bo[:, kt, :, :, :].rearrange(
                                    "s p u two -> p s u two"))
                        for kt in range(4):
                            ys = ftp.tile([P, H], BF)
                            yd = ftp.tile([P, H], BF)
                            ysv = ys[:].rearrange("p (s u) -> p s u", s=NCORES)
                            ydv = yd[:].rearrange("p (s u) -> p s u", s=NCORES)
                            nc.vector.tensor_tensor(
                                ysv, yt[:, kt, :, :, 0], yt[:, kt, :, :, 1],
                                ADD)
                            nc.vector.tensor_tensor(
                                ydv, yt[:, kt, :, :, 0], yt[:, kt, :, :, 1],
                                SUB)
                            for t2 in range(16):
                                tsl = slice(t2 * P, (t2 + 1) * P)
                                pts = pst.tile([P, P], BF)
                                nc.tensor.transpose(pts[:], ys[:, tsl], eye[:])
                                nc.vector.tensor_copy(
                                    ysT[:, t2, kt * P:(kt + 1) * P], pts[:])
                                ptd = pst.tile([P, P], BF)
                                nc.tensor.transpose(ptd[:], yd[:, tsl], eye[:])
                                nc.vector.tensor_copy(
                                    ydT[:, t2, kt * P:(kt + 1) * P], ptd[:])

                    # ----- stage 2 matmuls -----
                    with (
                        tc.tile_pool(name="g0sl", bufs=2) as g0pool,
                        tc.tile_pool(name="w0tmp", bufs=3) as w0tpool,
                        tc.tile_pool(name="w0pool", bufs=3) as w0pool,
                        tc.tile_pool(name="opool", bufs=4) as opool,
                        tc.tile_pool(name="pso", bufs=1, space="PSUM") as pso,
                        tc.tile_pool(name="psw0", bufs=2, space="PSUM") as psw0,
                    ):
                        for par in range(2):
                            src = ysT if par == 0 else ydT
                            for g in range(4):
                                lsl = slice(g * KC, (g + 1) * KC)
                                gk = g0pool.tile([1, KC], FP32, name='g0k')
                                gm = g0pool.tile([1, KC], FP32, name='g0m')
                                gf = g0pool.tile([1, KC], FP32, name='g0f')
                                nc.sync.dma_start(
                                    gk[:], g0_d[par:par + 1, lsl])
                                nc.sync.dma_start(
                                    gm[:], g0_d[2 + par:3 + par, lsl])
                                nc.sync.dma_start(
                                    gf[:], g0_d[4 + par:5 + par, lsl])
                                po = [pso.tile([P, KC], FP32, name=f'po{lt}')
                                      for lt in range(4)]
                                for t2 in range(16):
                                    pw = psw0.tile([P, KC], FP32)
                                    nc.tensor.matmul(
                                        pw[:], jj[0][t2][:], gk[:],
                                        start=True, stop=False)
                                    nc.tensor.matmul(
                                        pw[:], jj[1][t2][:], gm[:],
                                        start=False, stop=False)
                                    nc.tensor.matmul(
                                        pw[:], one[:], gf[:],
                                        start=False, stop=True)
                                    wi = w0tpool.tile([P, KC], I32,
                                                      name='w0i')
                                    nc.vector.tensor_copy(wi[:], pw[:])
                                    wi2 = w0tpool.tile([P, KC], I32,
                                                       name='w0i2')
                                    nc.vector.tensor_scalar(
                                        wi2[:], wi[:], N - 1, None, AND)
                                    wb = w0pool.tile([P, KC], BF)
                                    nc.scalar.activation(
                                        wb[:], wi2[:], SIN, bias=bias_t[:],
                                        scale=SC)
                                    mv = src[:, t2, :]
                                    for lt in range(4):
                                        nc.tensor.matmul(
                                            po[lt][:],
                                            wb[:, lt * P:(lt + 1) * P],
                                            mv, start=(t2 == 0),
                                            stop=(t2 == 15))
                                for lt in range(4):
                                    ob = opool.tile([P, KC], BF)
                                    idx = par * 16 + g * 4 + lt
                                    nc.vector.tensor_scalar(
                                        ob[:], po[lt][:], r0[:, idx:idx + 1],
                                        None, MULT)
                                    q0 = KC * g + P * lt
                                    nc.sync.dma_start(
                                        out_d[q0:q0 + P, par, :], ob[:])

    nc.compile()
    return nc


def _get_nc(reps=1):
    key = f"nc{reps}"
    if key not in _NC_CACHE:
        _NC_CACHE[key] = _build_nc(reps)
    return _NC_CACHE[key]


def _make_in_maps(x, expk0, expk1):
    return _prep(x, expk0, expk1)


def _assemble(results):
    return np.concatenate(
        [np.asarray(results[c]["out"]).reshape(N, KC).astype(np.float32)
         for c in range(NCORES)], axis=1)


def kernel(x, expk0, expk1):
    from concourse.bass_utils import run_bass_kernel_spmd

    in_maps = _prep(x, expk0, expk1)
    nc = _get_nc()
    try:
        res = run_bass_kernel_spmd(nc, in_maps, core_ids=list(range(NCORES)))
    except Exception:
        res = run_bass_kernel_spmd(nc, in_maps, core_ids=list(range(NCORES)))
    return _assemble(res.results)


if __name__ == "__main__":
    import sys
    import jax
    jax.config.update("jax_default_device", jax.devices("cpu")[0])
    import reference

    inputs = reference.setup_inputs()
    x = np.asarray(inputs["x"])
    e0 = np.asarray(inputs["expk0"])
    e1 = np.asarray(inputs["expk1"])
    expected = np.asarray(reference.reference(**inputs))

    if "--sim" in sys.argv:
        from concourse.bass_interp import MultiCoreSim
        nc = _get_nc()
        in_maps = _prep(x, e0, e1)
        sim = MultiCoreSim(nc, num_cores=NCORES)
        cores = list(sim.cores.values())
        for c, cs in enumerate(cores):
            for name, val in in_maps[c].items():
                cs.tensor(name)[:] = val
        sim.simulate(check_with_hw=False)
        results = [{"out": cores[c].tensor("out")} for c in range(NCORES)]
        got = _assemble(results)
    else:
        got = kernel(x, e0, e1)
    err = np.max(np.abs(got - expected)) / np.max(np.abs(expected))
    print(f"rel err: {err:.3e}")

